# revision 20
# baseline (speedup 1.0000x reference)
"""Trainium2 Bass kernel for nn_DynamicKernelSelection (moe_routing).

Strategy
--------
Host (cheap, O(B*C)):
  * Gating in float64 (argmax margins are far above fp32 noise).
  * Samples are paired by identical (expert1, expert2); at most one
    leftover pair can mix experts (pigeonhole) -- the device then runs that
    pair with slot-0's experts and the slot-1 sample is recomputed on the
    host (fp64, tiny fraction of total work).
  * Depthwise conv -> banded Toeplitz lhsT per (channel, kernel column):
    T[h', h] = W[c, dh, dw] at h' = h + (dh-k//2)*dil.  The H-taps become a
    single 128x128 matmul per kernel column; W-shifts are realized by
    accumulating the k_w matmuls into PSUM at shifted column ranges.

Device (SPMD, 8 cores; all fp16 single-pass -- tolerance is 2e-2, fp16
single-pass error is ~1e-3):
  * Work unit = (channel, pair): both samples of a pair are interleaved in
    the free dim as (w, s) so one N=256 matmul covers the pair.
  * Channels split 128/8 across cores; every core runs 8 pairs x 16
    channels -> uniform instruction stream and perfect load balance.
  * Channel-major loop: per channel, ONE x DMA (all pairs), ONE Toeplitz
    DMA (all experts, reused by all pairs -- 8x less traffic than
    per-pair), then k1 matmuls -> PSUM -> bias evac to fp16 (VectorE)
    per pair, stage-2 k2 matmuls on the fp16 out1 tile -> evac -> out2.
  * out1/out2 leave the device as fp16 (halves write traffic); the host
    upcasts to fp32.
"""

import numpy as np

B, C, H, W = 16, 128, 128, 128
N_CORES = 8
CPC = C // N_CORES           # channels per core (16)
NPAIR = B // 2               # 8 pairs
DIL1, DIL2 = 1, 3
K1S = {0: 3, 1: 5}           # stage-1 expert -> kernel size
K2S = {0: 7, 1: 9, 2: 11}

_PROGS = {}                  # signature -> compiled program


# --------------------------------------------------------------- host math
def _gating(x, aw1, ab1, aw2, ab2):
    pooled = x.astype(np.float64).mean(axis=(2, 3))
    l1 = pooled @ aw1.astype(np.float64).T + ab1.astype(np.float64)
    l2 = pooled @ aw2.astype(np.float64).T + ab2.astype(np.float64)
    return l1.argmax(axis=1), l2.argmax(axis=1)


def _band(wk, dil):
    """wk: [C, k, k] fp32 -> banded lhsT stack [C, H, k*H] fp16."""
    k = wk.shape[-1]
    t = np.zeros((C, H, k, H), np.float32)
    tv = t.transpose(1, 3, 0, 2)  # [h', h, C, dw] view
    c0 = k // 2
    for dh in range(k):
        d = (dh - c0) * dil
        h = np.arange(max(0, -d), H - max(0, d))
        tv[h + d, h] = wk[:, dh, :]
    return np.ascontiguousarray(t.reshape(C, H, k * H).astype(np.float16))


def _host_conv(x, wk, b, dil):
    """x [C,H,W] fp64, wk [C,k,k], b [C]: same-padded depthwise conv."""
    k = wk.shape[-1]
    c0 = k // 2
    out = np.zeros_like(x)
    for dh in range(k):
        for dw in range(k):
            dh_, dw_ = (dh - c0) * dil, (dw - c0) * dil
            hs = slice(max(0, -dh_), H - max(0, dh_))
            ws = slice(max(0, -dw_), W - max(0, dw_))
            hs2 = slice(max(0, dh_), H - max(0, -dh_))
            ws2 = slice(max(0, dw_), W - max(0, -dw_))
            out[:, hs, ws] += wk[:, dh, dw][:, None, None] * x[:, hs2, ws2]
    return out + b[:, None, None]


def _pair_samples(idx1, idx2):
    """Pair samples by (e1, e2); leftovers paired preferring same e1.
    Returns pairs [(a, b)] and fixes {sample: 'stage2' | 'both'}."""
    from collections import defaultdict
    groups = defaultdict(list)
    for s in range(B):
        groups[(int(idx1[s]), int(idx2[s]))].append(s)
    pairs, singles = [], []
    for key in sorted(groups):
        lst = groups[key]
        while len(lst) >= 2:
            pairs.append((lst.pop(0), lst.pop(0)))
        if lst:
            singles.append(lst[0])
    fixes = {}
    while singles:
        a = singles.pop(0)
        bi = next((i for i, s in enumerate(singles)
                   if idx1[s] == idx1[a]), 0)
        b = singles.pop(bi)
        pairs.append((a, b))
        fixes[b] = "stage2" if idx1[b] == idx1[a] else "both"
    return pairs, fixes


# ------------------------------------------------------------ device program
def _build_program(sig):
    """sig: ((k1, k2) per pair, stage-1 ks, stage-2 ks, mixed-pair index).

    The Toeplitz pack per channel is laid out as the stage-1 expert
    matrices (in e1ks order) followed by stage-2 (e2ks order).  The mixed
    pair's slot-1 stage-2 result is host-recomputed, so its stage-2 runs
    at N=W on the compacted slot-0 lanes only."""
    import concourse.tile as tile
    from concourse import bacc, mybir

    pair_ks, e1ks, e2ks, mix_p = sig
    dt = mybir.dt.float32
    f16 = mybir.dt.float16
    add = mybir.AluOpType.add
    nc = bacc.Bacc("TRN2", target_bir_lowering=False, debug=False,
                   enable_asserts=False, num_devices=N_CORES)

    # Toeplitz column offsets (in elements) per kernel size, pack order
    toff = {}
    off = 0
    for k in e1ks:
        toff[(1, k)] = off
        off += k * H
    for k in e2ks:
        toff[(2, k)] = off
        off += k * H
    TOT = off

    NW = NPAIR * 2 * W  # free-dim elements per channel (2048)
    x_d = nc.dram_tensor("x", [CPC, H, NW], f16, kind="ExternalInput").ap()
    t_d = nc.dram_tensor("t", [CPC, H, TOT], f16, kind="ExternalInput").ap()
    b_d = nc.dram_tensor("b", [H, CPC * NPAIR * 2], dt,
                         kind="ExternalInput").ap()
    o1_d = nc.dram_tensor("o1", [CPC, H, NW], f16, kind="ExternalOutput").ap()
    o2_d = nc.dram_tensor("o2", [CPC, H, NW], f16, kind="ExternalOutput").ap()

    def conv_mms(psum, tt, tbase, src, sbase, k, dil, il=2):
        c0 = k // 2
        order = [c0] + [dw for dw in range(k) if dw != c0]
        for j, dw in enumerate(order):
            d = (dw - c0) * dil
            a = max(0, -d)
            ln = W - abs(d)
            nc.tensor.matmul(
                out=psum[:, il * a:il * (a + ln)],
                lhsT=tt[:, tbase + dw * H:tbase + (dw + 1) * H],
                rhs=src[:, sbase + il * (a + d):sbase + il * (a + d + ln)],
                start=(j == 0), stop=(j == len(order) - 1),
                skip_group_check=True)

    with tile.TileContext(nc) as tc:
        with (tc.tile_pool(name="xp", bufs=3) as xp,
              tc.tile_pool(name="tp", bufs=3) as tp,
              tc.tile_pool(name="o1p", bufs=3) as o1p,
              tc.tile_pool(name="o2p", bufs=3) as o2p,
              tc.tile_pool(name="bp", bufs=1) as bp,
              tc.tile_pool(name="ps", bufs=8, space="PSUM") as ps):
            # PE warm-up: ~5us of tiny matmuls on a memset tile so the HAM
            # clock-gate is released before the first real matmul arrives.
            wt = bp.tile([128, 128], f16, tag="warm")
            nc.gpsimd.memset(wt[:], 0.0)
            pw = ps.tile([128, 2 * W], dt, tag="ps")
            for _ in range(80):
                nc.tensor.matmul(out=pw[:, 0:64], lhsT=wt[:], rhs=wt[:, 0:64],
                                 start=True, stop=True, skip_group_check=True)

            # stage-1 vs stage-2 halves of the Toeplitz pack (split the DMA
            # so the first matmuls' dependencies land first)
            T1C = sum(k * H for k in e1ks)
            k1f = pair_ks[0][0]
            f0, f1 = toff[(1, k1f)], toff[(1, k1f)] + k1f * H
            bt = None
            for u in range(CPC):
                xt = xp.tile([128, NW], f16, tag="x")
                tt = tp.tile([128, TOT], f16, tag="t")
                if u == 0:
                    # land pair 0's dependencies first, then bias (needed by
                    # the first evacs), then the rest
                    nc.sync.dma_start(out=xt[:, 0:2 * W], in_=x_d[u][:, 0:2 * W])
                    nc.sync.dma_start(out=tt[:, f0:f1], in_=t_d[u][:, f0:f1])
                    bt = bp.tile([128, CPC * NPAIR * 2], dt, tag="b")
                    nc.sync.dma_start(out=bt[:], in_=b_d)
                    nc.sync.dma_start(out=xt[:, 2 * W:8 * W],
                                      in_=x_d[u][:, 2 * W:8 * W])
                    if f0 > 0:
                        nc.sync.dma_start(out=tt[:, 0:f0], in_=t_d[u][:, 0:f0])
                    if f1 < T1C:
                        nc.sync.dma_start(out=tt[:, f1:T1C],
                                          in_=t_d[u][:, f1:T1C])
                    nc.sync.dma_start(out=xt[:, 8 * W:], in_=x_d[u][:, 8 * W:])
                else:
                    nc.sync.dma_start(out=xt[:], in_=x_d[u])
                    nc.sync.dma_start(out=tt[:, 0:T1C], in_=t_d[u][:, 0:T1C])
                nc.sync.dma_start(out=tt[:, T1C:TOT], in_=t_d[u][:, T1C:TOT])
                o1t = o1p.tile([128, NW], f16, tag="o1")
                o2t = o2p.tile([128, NW], f16, tag="o2")
                HP = NPAIR // 2
                for p, (k1, k2) in enumerate(pair_ks):
                    cols = slice(p * 2 * W, (p + 1) * 2 * W)
                    p1 = ps.tile([128, 2 * W], dt, tag="ps")
                    conv_mms(p1, tt, toff[(1, k1)], xt, p * 2 * W, k1, DIL1)
                    nc.vector.tensor_scalar(
                        out=o1t[:, cols], in0=p1[:],
                        scalar1=bt[:, (u * NPAIR + p) * 2:(u * NPAIR + p) * 2 + 1],
                        scalar2=None, op0=add)
                    if p == HP - 1:
                        nc.sync.dma_start(out=o1_d[u][:, :HP * 2 * W],
                                          in_=o1t[:, :HP * 2 * W])
                nc.sync.dma_start(out=o1_d[u][:, HP * 2 * W:],
                                  in_=o1t[:, HP * 2 * W:])
                for p, (k1, k2) in enumerate(pair_ks):
                    cols = slice(p * 2 * W, (p + 1) * 2 * W)
                    bs2 = bt[:, (u * NPAIR + p) * 2 + 1:(u * NPAIR + p) * 2 + 2]
                    if p == mix_p:
                        # slot-1 is host-recomputed: compact slot-0 lanes,
                        # run stage-2 at N=W, write both lanes (slot-1 lane
                        # is junk-but-initialized; host overwrites it).
                        o1m = o1p.tile([128, W], f16, tag="o1m")
                        nc.vector.tensor_copy(
                            out=o1m[:], in_=o1t[:, p * 2 * W:(p + 1) * 2 * W:2])
                        p2 = ps.tile([128, 2 * W], dt, tag="ps")
                        conv_mms(p2, tt, toff[(2, k2)], o1m, 0, k2, DIL2, il=1)
                        nc.vector.tensor_scalar(
                            out=o2t[:, p * 2 * W:(p + 1) * 2 * W:2],
                            in0=p2[:, 0:W],
                            scalar1=bs2, scalar2=None, op0=add)
                        nc.vector.tensor_scalar(
                            out=o2t[:, p * 2 * W + 1:(p + 1) * 2 * W:2],
                            in0=p2[:, 0:W], scalar1=bs2, scalar2=None, op0=add)
                    else:
                        p2 = ps.tile([128, 2 * W], dt, tag="ps")
                        conv_mms(p2, tt, toff[(2, k2)], o1t, p * 2 * W, k2, DIL2)
                        nc.vector.tensor_scalar(
                            out=o2t[:, cols], in0=p2[:],
                            scalar1=bs2, scalar2=None, op0=add)
                    if p % 2 == 1:
                        qc = slice((p - 1) * 2 * W, (p + 1) * 2 * W)
                        nc.sync.dma_start(out=o2_d[u][:, qc], in_=o2t[:, qc])
    nc.compile()
    return nc


# ------------------------------------------------------------------- driver
def kernel(x, aw1, ab1, aw2, ab2, w1_3, b1_3, w1_5, b1_5,
           w2_7, b2_7, w2_9, b2_9, w2_11, b2_11):
    from concourse.bass_utils import run_bass_kernel_spmd

    x = np.ascontiguousarray(np.asarray(x, dtype=np.float32))
    assert x.shape == (B, C, H, W)

    idx1, idx2 = _gating(np.asarray(x), np.asarray(aw1), np.asarray(ab1),
                         np.asarray(aw2), np.asarray(ab2))
    pairs, fixes = _pair_samples(idx1, idx2)

    w1e = [np.ascontiguousarray(np.asarray(w, np.float32)[:, 0])
           for w in (w1_3, w1_5)]
    w2e = [np.ascontiguousarray(np.asarray(w, np.float32)[:, 0])
           for w in (w2_7, w2_9, w2_11)]
    b1e = [np.asarray(b, np.float32) for b in (b1_3, b1_5)]
    b2e = [np.asarray(b, np.float32) for b in (b2_7, b2_9, b2_11)]

    # per-pair experts = slot-0's selection
    pe1 = [int(idx1[a]) for a, _ in pairs]
    pe2 = [int(idx2[a]) for a, _ in pairs]
    e1ks = tuple(sorted({K1S[e] for e in pe1}))
    e2ks = tuple(sorted({K2S[e] for e in pe2}))
    pair_ks = tuple((K1S[e1], K2S[e2]) for e1, e2 in zip(pe1, pe2))
    mix_p = next((i for i, (a, b) in enumerate(pairs) if b in fixes), -1)
    sig = (pair_ks, e1ks, e2ks, mix_p)

    if sig not in _PROGS:
        _PROGS[sig] = _build_program(sig)
    nc = _PROGS[sig]

    # fp16 banded lhsT per distinct expert, packed [C, H, TOT]
    packs = []
    for e in sorted({e for e in pe1}, key=lambda e: K1S[e]):
        packs.append(_band(w1e[e], DIL1))
    for e in sorted({e for e in pe2}, key=lambda e: K2S[e]):
        packs.append(_band(w2e[e], DIL2))
    tpack = np.concatenate(packs, axis=2)  # [C, H, TOT]

    # x packed [C, H, NPAIR*2W] fp16: per pair interleaved (w, s)
    ab = np.array(pairs)                     # [NPAIR, 2]
    xsel = x[ab.reshape(-1)].reshape(NPAIR, 2, C, H, W)
    xpk = np.ascontiguousarray(
        xsel.transpose(2, 3, 0, 4, 1), dtype=np.float16).reshape(C, H, -1)

    # biases [H, CPC*NPAIR*2] per core (broadcast along H)
    bsel = np.empty((CPC, NPAIR, 2), np.float32)

    in_maps = []
    for core in range(N_CORES):
        cs = slice(core * CPC, (core + 1) * CPC)
        for ui, c in enumerate(range(core * CPC, (core + 1) * CPC)):
            for p, (e1, e2) in enumerate(zip(pe1, pe2)):
                bsel[ui, p, 0] = b1e[e1][c]
                bsel[ui, p, 1] = b2e[e2][c]
        m = {"x": xpk[cs], "t": tpack[cs],
             "b": np.ascontiguousarray(
                 np.broadcast_to(bsel.reshape(1, -1), (H, CPC * NPAIR * 2)))}
        in_maps.append(m)

    res = run_bass_kernel_spmd(nc, in_maps, list(range(N_CORES)))

    out1 = np.empty((B, C, H, W), np.float32)
    out2 = np.empty((B, C, H, W), np.float32)
    for core in range(N_CORES):
        cs = slice(core * CPC, (core + 1) * CPC)
        r = res.results[core]
        o1 = r["o1"].reshape(CPC, H, NPAIR, W, 2).astype(np.float32)
        o2 = r["o2"].reshape(CPC, H, NPAIR, W, 2).astype(np.float32)
        for p, (a, b) in enumerate(pairs):
            out1[a, cs] = o1[:, :, p, :, 0]
            out1[b, cs] = o1[:, :, p, :, 1]
            out2[a, cs] = o2[:, :, p, :, 0]
            out2[b, cs] = o2[:, :, p, :, 1]

    # host fix-up for mixed pairs (at most 2 samples)
    for s, kind in fixes.items():
        e1, e2 = int(idx1[s]), int(idx2[s])
        if kind == "both":
            o1 = _host_conv(x[s].astype(np.float64), w1e[e1].astype(np.float64),
                            b1e[e1].astype(np.float64), DIL1)
            out1[s] = o1.astype(np.float32)
        else:
            o1 = out1[s].astype(np.float64)
        out2[s] = _host_conv(o1, w2e[e2].astype(np.float64),
                             b2e[e2].astype(np.float64), DIL2).astype(np.float32)
    return out1, out2
